# revision 28
# baseline (speedup 1.0000x reference)
"""Trainium2 Bass kernel for a 3x3 stride-1 pad-1 conv, NCHW (16,16,512,512) fp32.

Matches the reference semantics exactly:
  - effective weights: K flattened as (ki,kj,ci) but consumed as (ci,ki,kj):
      Weff[ki,kj,ci,co] = K.reshape(144,16)[ci*9 + ki*3 + kj, co]
  - last output row and column are zero (applied host-side).

Strategy: pure data parallel over the batch (2 images per core on 8 cores),
weights replicated.

The conv runs as banded fp16 matmuls: output rows in groups of R=6, with
contraction K = 8 input rows x 16 c_in = 128 partitions and M = 6 out rows x
16 c_out = 96; the 3 kj taps are column-shifted rhs slices accumulated in
PSUM (3 matmuls of N=512 per group, issued kj-major over subrounds of 4
groups so consecutive matmuls hit different PSUM banks and reuse weights).

All heavy data movement is structured around the DMA engines (the original
bottleneck: HWDGE queues only fan out to 6-8 of the 16 DMA engines, and
NCHW-layout tiles produce 1KB descriptors):
  - x is staged host-side in fp16 and PRE-PERMUTED into the exact SBUF tile
    layout: [img, megatile, 128 partitions, 8 groups, 514 cols] with the
    conv zero-padding baked in.  A megatile load is one DMA whose
    per-partition descriptor is 8x1028B contiguous, and consecutive
    partitions are DRAM-contiguous (SWDGE aggregates packets).
  - the output is stored as fp16 in a permuted layout [img, subround, 96
    partitions, 4 groups, 512] (halves write traffic vs fp32 NCHW) and
    un-permuted + cast to fp32 on the host.
  - every bulk DMA is issued on gpsimd (SWDGE, qPoolDynamic) because that
    queue round-robins over all 16 DMA engines; HWDGE rings concentrate on
    engines 64-71.
  - the final tiles store in per-subround / per-group chunks so the drain
    after the last matmul is short.
PSUM->SBUF copies (with the fp32->fp16 cast) alternate between the DVE and
Activation engines so neither becomes the bottleneck.
"""

import numpy as np

import concourse.bass as bass
import concourse.mybir as mybir
import concourse.tile as tile
from concourse import bacc
from concourse.bass_utils import run_bass_kernel_spmd

F32 = mybir.dt.float32
F16 = mybir.dt.float16

C = 16     # channels (in == out)
W = 512    # image width
H = 512    # image height
R = 6      # output rows per matmul group
RIN = R + 2  # input rows per group
M = R * C   # matmul output partitions (96)
GW = W + 2  # staged cols per group: input cols -1..512
GT = 8      # group slots per megatile
NT = 11     # megatiles per image (10 full + 1 with 6 groups)
NSR = 2 * NT  # store subrounds per image (4 groups each, tail has 2)
N_IMG = 2   # images per core
N_CORES = 8

# group start rows: out rows of group g are S[g]..S[g]+5
S = [6 * g for g in range(85)] + [505]  # 86 groups, out rows 0..510
N_GROUPS = len(S)


def _weff(K: np.ndarray) -> np.ndarray:
    Kflat = K.reshape(9 * C, C).astype(np.float32)
    Weff = np.zeros((3, 3, C, C), np.float32)
    for ki in range(3):
        for kj in range(3):
            for ci in range(C):
                Weff[ki, kj, ci, :] = Kflat[ci * 9 + ki * 3 + kj, :]
    return Weff


def _build_banded_weights(K: np.ndarray) -> np.ndarray:
    """lhsT matrices [128, 3, 96] fp16; k = hi*16+ci, m = ho*16+co, ki=hi-ho.
    Stored partition-major so the weight load is one 576B-per-partition DMA."""
    Weff = _weff(K)
    wa = np.zeros((128, 3, M), np.float32)
    for kj in range(3):
        for ho in range(R):
            for ki in range(3):
                hi = ho + ki
                blk = Weff[ki, kj]  # [ci, co]
                for ci in range(C):
                    wa[hi * C + ci, kj, ho * C:(ho + 1) * C] = blk[ci]
    return wa.astype(np.float16)


def _stage_inputs(x16: np.ndarray) -> np.ndarray:
    """[B, C, H, W] fp16 -> [B, NT, 128, GT, GW] fp16 banded-group layout.

    Partition p = hi*16+ci of group g holds input row S[g]-1+hi (row -1 and
    the left/right pad columns are zeros, baked in here)."""
    B = x16.shape[0]
    xpad = np.zeros((B, C, H + 1, GW), np.float16)
    xpad[:, :, 1:, 1:W + 1] = x16  # row r at index r+1, col c at index c+1
    idx = np.asarray(S)[:, None] + np.arange(RIN)[None, :]  # [86, 8] = S[g]+hi
    g = xpad[:, :, idx, :]              # [B, C, 86, 8, GW]
    g = g.transpose(0, 2, 3, 1, 4)      # [B, 86, hi, ci, GW]
    g = g.reshape(B, N_GROUPS, 128, GW)
    out = np.zeros((B, NT * GT, 128, GW), np.float16)
    out[:, :N_GROUPS] = g
    out = out.reshape(B, NT, GT, 128, GW).transpose(0, 1, 3, 2, 4)
    return np.ascontiguousarray(out)


def _unstage_output(perm: np.ndarray) -> np.ndarray:
    """[B, NSR, 96, 4, W] fp16 -> [B, C, H, W] fp32 with last row/col zeroed."""
    B = perm.shape[0]
    p = perm.transpose(0, 1, 3, 2, 4)      # [B, sr, slot, 96, col]
    p = p.reshape(B, NSR * 4, R, C, W)     # [B, group slot, ho, co, col]
    y = np.zeros((B, C, H, W), np.float32)
    reg = p[:, :85].transpose(0, 3, 1, 2, 4).reshape(B, C, 510, W)
    y[:, :, 0:510, :] = reg.astype(np.float32)
    y[:, :, 510, :] = p[:, 85, 5].astype(np.float32)  # out row 510
    y[:, :, :, W - 1] = 0.0  # masked last column (row 511 already zero)
    return y


def build_nc(in_bufs: int = 6, out_bufs: int = 4, psum_bufs: int = 8,
             lookahead: int = 4):
    nc = bacc.Bacc(None, target_bir_lowering=False)
    xs = nc.dram_tensor("xs", [N_IMG, NT, 128, GT, GW], F16,
                        kind="ExternalInput")
    whi = nc.dram_tensor("whi", [128, 3, M], F16, kind="ExternalInput")
    ys = nc.dram_tensor("ys", [N_IMG, NSR, M, 4, W], F16,
                        kind="ExternalOutput")

    # megatiles in issue order: (img, tile idx, groups in tile)
    tiles = [(n, t, 6 if t == NT - 1 else GT)
             for n in range(N_IMG) for t in range(NT)]

    with tile.TileContext(nc) as tc:
        with (
            tc.tile_pool(name="wpool", bufs=1) as wpool,
            tc.tile_pool(name="inpool", bufs=in_bufs) as inpool,
            tc.tile_pool(name="outpool", bufs=out_bufs) as outpool,
            tc.tile_pool(name="psum", bufs=psum_bufs, space="PSUM") as psum_pool,
        ):
            whi_t = wpool.tile([128, 3, M], F16)
            nc.sync.dma_start(
                whi_t[:], bass.AP(whi, 0, [[3 * M, 128], [M, 3], [1, M]])
            )

            in_tiles = {}

            def load(i):
                n, t, G = tiles[i]
                tl = inpool.tile([128, GT, GW], F16, name=f"in_{n}_{t}",
                                 tag="in")
                base = (n * NT + t) * 128 * GT * GW
                # split the very first load so its matmuls start sooner; the
                # first two loads ride the (otherwise idle) sync HWDGE ring,
                # which starts up faster than SWDGE
                chunks = ((0, 2), (2, G)) if i == 0 else ((0, G),)
                eng = nc.sync if i < 2 else nc.gpsimd
                for lo, hi in chunks:
                    src = bass.AP(xs, base + lo * GW,
                                  [[GT * GW, 128], [GW, hi - lo], [1, GW]])
                    eng.dma_start(tl[:, lo:hi, :], src)
                in_tiles[i] = tl

            def compute_tile(i):
                """kj-major over subrounds of 4 groups; copies alternate
                DVE/Act.  Stores: one DMA per megatile (4KB descriptors via
                the [2,4,W] split)."""
                n, t, G = tiles[i]
                tl = in_tiles[i]
                out_t = outpool.tile([M, 2, 4, W], F16, name=f"out_{n}_{t}",
                                     tag="out")
                base = (n * NSR + 2 * t) * M * 4 * W
                for j in (0, 1):
                    gs = list(range(4 * j, min(4 * j + 4, G)))
                    ps = [
                        psum_pool.tile([M, W], F32, name=f"ps_{n}_{t}_{g}",
                                       tag="ps")
                        for g in gs
                    ]
                    for kj in range(3):
                        for k, g in enumerate(gs):
                            nc.tensor.matmul(
                                ps[k][:], whi_t[:, kj, :],
                                tl[:, g, kj:kj + W],
                                start=(kj == 0), stop=(kj == 2),
                            )
                    for k, g in enumerate(gs):
                        if g % 2 == 0:
                            nc.vector.tensor_copy(out_t[:, j, k, :], ps[k][:])
                        else:
                            nc.scalar.copy(out_t[:, j, k, :], ps[k][:])
                # the last two tiles store via the scalar HWDGE ring so the
                # final writes interleave with the SWDGE backlog instead of
                # queueing behind it
                seng = nc.scalar if i >= len(tiles) - 2 else nc.gpsimd
                if G == GT:
                    dst = bass.AP(
                        ys, base,
                        [[4 * W, M], [M * 4 * W, 2], [W, 4], [1, W]])
                    seng.dma_start(dst, out_t[:])
                else:  # tail tile: 4 + 2 groups
                    dst0 = bass.AP(ys, base, [[4 * W, M], [W, 4], [1, W]])
                    seng.dma_start(dst0, out_t[:, 0, :, :])
                    dst1 = bass.AP(ys, base + M * 4 * W,
                                   [[4 * W, M], [W, 2], [1, W]])
                    seng.dma_start(dst1, out_t[:, 1, 0:2, :])

            for i in range(min(lookahead, len(tiles))):
                load(i)
            for i in range(len(tiles)):
                if i + lookahead < len(tiles):
                    load(i + lookahead)
                compute_tile(i)
                del in_tiles[i]

    nc.finalize()
    return nc


def _run(x: np.ndarray, K: np.ndarray, core_ids, trace=False, **kw):
    """x: [n_total, C, H, W] fp32, split evenly over core_ids."""
    n_cores = len(core_ids)
    n_total = x.shape[0]
    assert n_total % n_cores == 0 and n_total // n_cores == N_IMG
    wa = _build_banded_weights(K)
    x16 = x.astype(np.float16)
    staged = _stage_inputs(x16)  # [n_total, NT, 128, GT, GW]
    nc = build_nc(**kw)
    in_maps = [
        {
            "xs": np.ascontiguousarray(staged[i * N_IMG:(i + 1) * N_IMG]),
            "whi": wa,
        }
        for i in range(n_cores)
    ]
    res = run_bass_kernel_spmd(nc, in_maps, core_ids=list(core_ids),
                               trace=trace)
    perm = np.concatenate([r["ys"] for r in res.results], axis=0)
    y = _unstage_output(perm)
    return y, res


def kernel(**inputs) -> np.ndarray:
    x = np.ascontiguousarray(np.asarray(inputs["x"], dtype=np.float32))
    K = np.ascontiguousarray(np.asarray(inputs["K"], dtype=np.float32))
    y, _ = _run(x, K, core_ids=range(N_CORES))
    return y


# revision 31
# speedup vs baseline: 1.2328x; 1.2328x over previous
"""Trainium2 Bass kernel for a 3x3 stride-1 pad-1 conv, NCHW (16,16,512,512) fp32.

Matches the reference semantics exactly:
  - effective weights: K flattened as (ki,kj,ci) but consumed as (ci,ki,kj):
      Weff[ki,kj,ci,co] = K.reshape(144,16)[ci*9 + ki*3 + kj, co]
  - last output row and column are zero (applied host-side).

Strategy: pure data parallel over the batch (2 images per core on 8 cores),
weights replicated.

The conv runs as banded fp16 matmuls: output rows in groups of R=6, with
contraction K = 8 input rows x 16 c_in = 128 partitions and M = 6 out rows x
16 c_out = 96; the 3 kj taps are column-shifted rhs slices accumulated in
PSUM (3 matmuls of N=512 per group, issued kj-major over subrounds of 4
groups so consecutive matmuls hit different PSUM banks and reuse weights).

All heavy data movement is structured around the DMA engines (the original
bottleneck: HWDGE queues only fan out to 6-8 of the 16 DMA engines, and
NCHW-layout tiles produce 1KB descriptors):
  - x is staged host-side in fp16 and PRE-PERMUTED into the exact SBUF tile
    layout: [img, megatile, 128 partitions, 8 groups, 514 cols] with the
    conv zero-padding baked in.  A megatile load is one DMA whose
    per-partition descriptor is 8x1028B contiguous, and consecutive
    partitions are DRAM-contiguous (SWDGE aggregates packets).
  - the output is stored as fp16 in a permuted layout [img, subround, 96
    partitions, 4 groups, 512] (halves write traffic vs fp32 NCHW) and
    un-permuted + cast to fp32 on the host.
  - every bulk DMA is issued on gpsimd (SWDGE, qPoolDynamic) because that
    queue round-robins over all 16 DMA engines; HWDGE rings concentrate on
    engines 64-71.
  - the final tiles store in per-subround / per-group chunks so the drain
    after the last matmul is short.
PSUM->SBUF copies (with the fp32->fp16 cast) alternate between the DVE and
Activation engines so neither becomes the bottleneck.
"""

import numpy as np

import concourse.bass as bass
import concourse.mybir as mybir
import concourse.tile as tile
from concourse import bacc
from concourse.bass_utils import run_bass_kernel_spmd

F32 = mybir.dt.float32
F16 = mybir.dt.float16

C = 16     # channels (in == out)
W = 512    # image width
H = 512    # image height
R = 6      # output rows per matmul group
RIN = R + 2  # input rows per group
M = R * C   # matmul output partitions (96)
GW = W + 2  # staged cols per group: input cols -1..512
GT = 8      # group slots per megatile
NT = 11     # megatiles per image (10 full + 1 with 6 groups)
NSR = 2 * NT  # store subrounds per image (4 groups each, tail has 2)
N_IMG = 2   # images per core
N_CORES = 8

# group start rows: out rows of group g are S[g]..S[g]+5
S = [6 * g for g in range(85)] + [505]  # 86 groups, out rows 0..510
N_GROUPS = len(S)


def _weff(K: np.ndarray) -> np.ndarray:
    Kflat = K.reshape(9 * C, C).astype(np.float32)
    Weff = np.zeros((3, 3, C, C), np.float32)
    for ki in range(3):
        for kj in range(3):
            for ci in range(C):
                Weff[ki, kj, ci, :] = Kflat[ci * 9 + ki * 3 + kj, :]
    return Weff


def _build_banded_weights(K: np.ndarray) -> np.ndarray:
    """lhsT matrices [128, 3, 96] fp16; k = hi*16+ci, m = ho*16+co, ki=hi-ho.
    Stored partition-major so the weight load is one 576B-per-partition DMA."""
    Weff = _weff(K)
    wa = np.zeros((128, 3, M), np.float32)
    for kj in range(3):
        for ho in range(R):
            for ki in range(3):
                hi = ho + ki
                blk = Weff[ki, kj]  # [ci, co]
                for ci in range(C):
                    wa[hi * C + ci, kj, ho * C:(ho + 1) * C] = blk[ci]
    return wa.astype(np.float16)


def _stage_inputs(x16: np.ndarray) -> np.ndarray:
    """[B, C, H, W] fp16 -> [B, NT, 128, GT, GW] fp16 banded-group layout.

    Partition p = hi*16+ci of group g holds input row S[g]-1+hi (row -1 and
    the left/right pad columns are zeros, baked in here)."""
    B = x16.shape[0]
    xpad = np.zeros((B, C, H + 1, GW), np.float16)
    xpad[:, :, 1:, 1:W + 1] = x16  # row r at index r+1, col c at index c+1
    idx = np.asarray(S)[:, None] + np.arange(RIN)[None, :]  # [86, 8] = S[g]+hi
    g = xpad[:, :, idx, :]              # [B, C, 86, 8, GW]
    g = g.transpose(0, 2, 3, 1, 4)      # [B, 86, hi, ci, GW]
    g = g.reshape(B, N_GROUPS, 128, GW)
    out = np.zeros((B, NT * GT, 128, GW), np.float16)
    out[:, :N_GROUPS] = g
    out = out.reshape(B, NT, GT, 128, GW).transpose(0, 1, 3, 2, 4)
    return np.ascontiguousarray(out)


def _unstage_output(perm: np.ndarray) -> np.ndarray:
    """[B, NSR, 96, 4, W] fp16 -> [B, C, H, W] fp32 with last row/col zeroed."""
    B = perm.shape[0]
    p = perm.transpose(0, 1, 3, 2, 4)      # [B, sr, slot, 96, col]
    p = p.reshape(B, NSR * 4, R, C, W)     # [B, group slot, ho, co, col]
    y = np.zeros((B, C, H, W), np.float32)
    reg = p[:, :85].transpose(0, 3, 1, 2, 4).reshape(B, C, 510, W)
    y[:, :, 0:510, :] = reg.astype(np.float32)
    y[:, :, 510, :] = p[:, 85, 5].astype(np.float32)  # out row 510
    y[:, :, :, W - 1] = 0.0  # masked last column (row 511 already zero)
    return y


def build_nc(in_bufs: int = 6, out_bufs: int = 3, psum_bufs: int = 8,
             lookahead: int = 4):
    nc = bacc.Bacc(None, target_bir_lowering=False)
    xs = nc.dram_tensor("xs", [N_IMG, NT, 128, GT, GW], F16,
                        kind="ExternalInput")
    whi = nc.dram_tensor("whi", [128, 3, M], F16, kind="ExternalInput")
    ys = nc.dram_tensor("ys", [N_IMG, NSR, M, 4, W], F16,
                        kind="ExternalOutput")

    # megatiles in issue order: (img, tile idx, groups in tile)
    tiles = [(n, t, 6 if t == NT - 1 else GT)
             for n in range(N_IMG) for t in range(NT)]

    with tile.TileContext(nc) as tc:
        with (
            tc.tile_pool(name="wpool", bufs=1) as wpool,
            tc.tile_pool(name="inpool", bufs=in_bufs) as inpool,
            tc.tile_pool(name="outpool", bufs=out_bufs) as outpool,
            tc.tile_pool(name="psum", bufs=psum_bufs, space="PSUM") as psum_pool,
        ):
            whi_t = wpool.tile([128, 3, M], F16)
            nc.sync.dma_start(
                whi_t[:], bass.AP(whi, 0, [[3 * M, 128], [M, 3], [1, M]])
            )

            in_tiles = {}

            def load(i):
                n, t, G = tiles[i]
                tl = inpool.tile([128, GT, GW], F16, name=f"in_{n}_{t}",
                                 tag="in")
                base = (n * NT + t) * 128 * GT * GW
                # split the very first load so its matmuls start sooner
                chunks = ((0, 2), (2, G)) if i == 0 else ((0, G),)
                for lo, hi in chunks:
                    src = bass.AP(xs, base + lo * GW,
                                  [[GT * GW, 128], [GW, hi - lo], [1, GW]])
                    nc.gpsimd.dma_start(tl[:, lo:hi, :], src)
                in_tiles[i] = tl

            def compute_tile(i):
                """kj-major over subrounds of 4 groups; copies alternate
                DVE/Act.  Stores: one DMA per megatile (4KB descriptors via
                the [2,4,W] split)."""
                n, t, G = tiles[i]
                tl = in_tiles[i]
                out_t = outpool.tile([M, 2, 4, W], F16, name=f"out_{n}_{t}",
                                     tag="out")
                base = (n * NSR + 2 * t) * M * 4 * W
                for j in (0, 1):
                    gs = list(range(4 * j, min(4 * j + 4, G)))
                    ps = [
                        psum_pool.tile([M, W], F32, name=f"ps_{n}_{t}_{g}",
                                       tag="ps")
                        for g in gs
                    ]
                    for kj in range(3):
                        for k, g in enumerate(gs):
                            nc.tensor.matmul(
                                ps[k][:], whi_t[:, kj, :],
                                tl[:, g, kj:kj + W],
                                start=(kj == 0), stop=(kj == 2),
                            )
                    for k, g in enumerate(gs):
                        if g % 2 == 0:
                            nc.vector.tensor_copy(out_t[:, j, k, :], ps[k][:])
                        else:
                            nc.scalar.copy(out_t[:, j, k, :], ps[k][:])
                if G == GT:
                    dst = bass.AP(
                        ys, base,
                        [[4 * W, M], [M * 4 * W, 2], [W, 4], [1, W]])
                    nc.gpsimd.dma_start(dst, out_t[:])
                else:  # tail tile: 4 + 2 groups
                    dst0 = bass.AP(ys, base, [[4 * W, M], [W, 4], [1, W]])
                    nc.gpsimd.dma_start(dst0, out_t[:, 0, :, :])
                    dst1 = bass.AP(ys, base + M * 4 * W,
                                   [[4 * W, M], [W, 2], [1, W]])
                    nc.gpsimd.dma_start(dst1, out_t[:, 1, 0:2, :])

            for i in range(min(lookahead, len(tiles))):
                load(i)
            for i in range(len(tiles)):
                if i + lookahead < len(tiles):
                    load(i + lookahead)
                compute_tile(i)
                del in_tiles[i]

    nc.finalize()
    return nc


def _run(x: np.ndarray, K: np.ndarray, core_ids, trace=False, **kw):
    """x: [n_total, C, H, W] fp32, split evenly over core_ids."""
    n_cores = len(core_ids)
    n_total = x.shape[0]
    assert n_total % n_cores == 0 and n_total // n_cores == N_IMG
    wa = _build_banded_weights(K)
    x16 = x.astype(np.float16)
    staged = _stage_inputs(x16)  # [n_total, NT, 128, GT, GW]
    nc = build_nc(**kw)
    in_maps = [
        {
            "xs": np.ascontiguousarray(staged[i * N_IMG:(i + 1) * N_IMG]),
            "whi": wa,
        }
        for i in range(n_cores)
    ]
    res = run_bass_kernel_spmd(nc, in_maps, core_ids=list(core_ids),
                               trace=trace)
    perm = np.concatenate([r["ys"] for r in res.results], axis=0)
    y = _unstage_output(perm)
    return y, res


def kernel(**inputs) -> np.ndarray:
    x = np.ascontiguousarray(np.asarray(inputs["x"], dtype=np.float32))
    K = np.ascontiguousarray(np.asarray(inputs["K"], dtype=np.float32))
    y, _ = _run(x, K, core_ids=range(N_CORES))
    return y
